# revision 17
# baseline (speedup 1.0000x reference)
"""Trainium2 Bass kernel for nn_CGNN (gnn_message_passing).

Strategy
--------
The per-edge gather/scatter-add over a shared edge list is algebraically a
dense matmul: messages[b] = A @ h_new[b] with A[n, m] = sum_{e: dst=n, src=m}
w_e (A is [128, 128], shared across batch and layers).  The whole network is
then dense matmuls + relu, executed per-sample as [128, 256] tiles:

  - h is stored per-sample TRANSPOSED (hT: feature on partitions, node on
    free dim) in one resident SBUF buffer.  The layer matmul z = h @ W uses
    hT chunks as the stationary operand (lhsT), which re-transposes for free:
    z comes out in normal [node, feature] layout.
  - messages^T = h_new^T-producing matmul: lhsT = h_new (normal layout, from
    the relu eviction of z), rhs = A^T.  Output mT is in hT layout.
  - The residual h + messages is accumulated on the PE itself: an identity
    matmul adds hT into the mT PSUM accumulation; a single DVE
    relu-max eviction then produces the next layer's hT (fp16).
  - The encoder h0 = x*enc_w + enc_b is never materialized: layer 1 uses
    z1 = x (x) u + 1 (x) c  (u = enc_w @ W1, c = enc_b @ W1 + b1, computed on
    host in fp64).  Sample s's [1; x_s] row-pair lives at SBUF partitions
    (32k, 32k+1), k = s%3, col block s//3 of a compact [128, 11008] tile
    (rank-2 stationaries: K=2 matmuls).  h0's residual enters the layer-1
    mT PSUM as a K=2 matmul from [enc_b; enc_w] chunk rows at the same
    partition pair.
  - The classifier hidden = relu(h3.flat @ cls_w1 + b1) reads the resident
    hT buffer with strided APs (no transposes): for each 128-row chunk of
    cls_w1, rhs = hT[h-partitions, batch-strided free].  cls_w1 is streamed
    from HBM in fp16, prefetched during the layer phase, DMAs round-robined
    across the SP/Act/DVE queues.

Data-parallel across 8 cores over the batch axis (256 samples/core).
All matmuls in fp16 (fp32 PSUM accumulation).
"""

import sys

for _p in ("/opt/trn_rl_repo",):
    if _p not in sys.path:
        sys.path.insert(0, _p)

from contextlib import ExitStack

import ml_dtypes
import numpy as np

import concourse.bacc as bacc
import concourse.bass as bass
import concourse.tile as tile
from concourse import mybir
from concourse.bass_utils import run_bass_kernel_spmd

dt = mybir.dt
AF = mybir.ActivationFunctionType
F16 = np.float16

B, N, H, NL, OUT = 2048, 128, 256, 3, 2
N_CORES = 8
BC = B // N_CORES            # samples per core (256)
G = 2                        # samples per elementwise eviction group
NG = BC // G
KB = 8                       # cls_w1 128-row chunks per DMA (512 KB each)
N_CHUNKS = (N * H) // 128    # 256 contraction chunks in the classifier
W1_TILES = N_CHUNKS // KB    # 32
W1_PREFETCH = 4              # w1 tiles DMA'd before the layer loop ends

_BUILT = {}


def _build_nc(has_lbias: bool, bc: int = BC, mode: str = "full",
              repeat: int = 1):
    """Emit the Tile kernel. has_lbias: include the (rare) nonzero
    layer-bias rank-1 accumulations for layers 2..3.
    mode: "full" | "layers" (skip classifier, dump ht) | "cls"
    (skip layers, classifier reads zero-init ht)."""
    ng = bc // G
    n_chunks = N_CHUNKS
    nc = bacc.Bacc("TRN2", target_bir_lowering=False)

    # compact x: sample s -> partition pair (32k, 32k+1), k=s%3, col block
    # s//3.  Row 32k is all-ones, row 32k+1 is x_s.
    lhsx_d = nc.dram_tensor("lhsx", [6, ((bc + 2) // 3) * 128], dt.float16,
                            kind="ExternalInput")
    # packed fp16 const blob: one DMA covers everything the layer phase
    # needs beyond lhsx.  cols: [0:256] u2c ([c; u] at partition pairs),
    # [256:512] ew ([enc_b; enc_w] chunks at pairs), [512:640] at_t,
    # [640:768] eye, [768:772] w2r.
    ck_d = nc.dram_tensor("ck", [128, 772], dt.float16,
                          kind="ExternalInput")
    w23_d = nc.dram_tensor("w23", [NL - 1, H, H], dt.float16,
                           kind="ExternalInput")
    w1_d = nc.dram_tensor("w1", [N * H, H], dt.float16, kind="ExternalInput")
    cbb_d = nc.dram_tensor("cbb", [128, 4], dt.float32,
                           kind="ExternalInput")
    if has_lbias:
        # layer-l bias row at partitions 32k, block l-1
        xb_d = nc.dram_tensor("xb23", [6, (NL - 1) * H], dt.float16,
                              kind="ExternalInput")
    if mode in ("layers", "l0", "l1"):
        out_d = nc.dram_tensor("htdump", [128, bc * H], dt.float16,
                               kind="ExternalOutput")
    else:
        out_d = nc.dram_tensor("logits", [bc, OUT], dt.float32,
                               kind="ExternalOutput")

    with tile.TileContext(nc) as tc, ExitStack() as ctx:
        const = ctx.enter_context(tc.tile_pool(name="const", bufs=1))
        htp = ctx.enter_context(tc.tile_pool(name="ht", bufs=1))

        lhsx = const.tile([128, ((bc + 2) // 3) * 128], dt.float16)
        ck = const.tile([128, 772], dt.float16)
        w23 = const.tile([128, (NL - 1) * 2 * H], dt.float16)
        cbb = const.tile([128, 4], dt.float32)


        # critical path: 4 HWDGE issues total (first z needs lhsx pair 0 +
        # ck; pairs 1/2 right behind)
        nc.sync.dma_start(lhsx[0:2, :], lhsx_d[0:2, :])
        nc.sync.dma_start(ck[:], ck_d[:])
        nc.sync.dma_start(lhsx[32:34, :], lhsx_d[2:4, :])
        nc.sync.dma_start(lhsx[64:66, :], lhsx_d[4:6, :])
        # non-critical consts via SWDGE (Pool) - off the HWDGE device
        for li in range(NL - 1):
            for hc in range(2):
                nc.gpsimd.dma_start(
                    w23[:, (li * 2 + hc) * H:(li * 2 + hc + 1) * H],
                    w23_d[li, hc * 128:(hc + 1) * 128, :])
        nc.gpsimd.dma_start(cbb[:], cbb_d[:])
        if has_lbias:
            xb = const.tile([128, (NL - 1) * H], dt.float16)
            for k in range(3):
                nc.gpsimd.dma_start(xb[32 * k:32 * k + 1, :],
                                    xb_d[2 * k:2 * k + 1, :])

        # resident h (hT layout): sample s chunk hc at cols s*256 + hc*128
        ht = htp.tile([128, bc * H], dt.float16)

        n0 = (bc + 2) // 3
        n1 = (bc + 1) // 3

        def pkcb(j):
            # processing slot j -> (pair k, col block cb); original sample
            # 3*cb + k.  Pair-major so early slots only touch lhsx pair 0.
            if j < n0:
                return 0, j
            if j < n0 + n1:
                return 1, j - n0
            return 2, j - n0 - n1

        def lx_ap(s):
            k, cb = pkcb(s)
            return lhsx[32 * k:32 * k + 2, cb * 128:(cb + 1) * 128]

        if mode == "cls":
            nc.vector.memset(ht[:], 0.5)

        w1_queues = (nc.sync, nc.scalar)
        w1_tiles = []          # (tile, in-flight) FIFO for prefetch
        w1_issued = [0]

        def issue_w1(w1p, w1_v):
            mc = w1_issued[0]
            if mc >= W1_TILES:
                return
            w1t = w1p.tile([128, KB * H], dt.float16, name="w1t")
            w1_queues[mc % 2].dma_start(
                w1t[:].rearrange("p (a k) -> p a k", a=KB),
                w1_v[:, mc * KB:(mc + 1) * KB, :])
            w1_tiles.append(w1t)
            w1_issued[0] = mc + 1

        # ---------------- phase 1: 3 GNN layers ----------------
        for _rep in range(repeat):
         rep_ctx = ExitStack()
         w1_v = w1_d[:].rearrange("(a p) k -> p a k", p=128)
         w1p = None
         if mode in ("full", "cls"):
             w1p = rep_ctx.enter_context(
                 tc.tile_pool(name=f"w1p{_rep}", bufs=W1_PREFETCH + 2))
         if mode != "cls":
           with (
               tc.tile_pool(name="hn", bufs=5) as hnp,
               tc.tile_pool(name="zp", bufs=8 // G, space="PSUM") as zp,
               tc.tile_pool(name="mp", bufs=8 // G, space="PSUM") as mp,
           ):
               def emit_z(l, g):
                   z = zp.tile([128, G * H], dt.float32)
                   for si in range(G):
                       s = g * G + si
                       zs = z[:, si * H:(si + 1) * H]
                       if l == 0:
                           k = pkcb(s)[0]
                           nc.tensor.matmul(
                               zs, lx_ap(s),
                               ck[32 * k:32 * k + 2, 0:H],
                               start=True, stop=True)
                       else:
                           for hc in range(2):
                               last = (hc == 1) and not has_lbias
                               nc.tensor.matmul(
                                   zs,
                                   ht[:, s * H + hc * 128:s * H + (hc + 1) * 128],
                                   w23[:, ((l - 1) * 2 + hc) * H:
                                       ((l - 1) * 2 + hc + 1) * H],
                                   start=(hc == 0), stop=last)
                           if has_lbias:
                               k = pkcb(s)[0]
                               nc.tensor.matmul(
                                   zs, lx_ap(s)[0:1, :],
                                   xb[32 * k:32 * k + 1,
                                      (l - 1) * H:l * H],
                                   start=False, stop=True)
                   hn = hnp.tile([128, G * H], dt.float16)
                   nc.scalar.activation(hn[:], z[:], AF.Relu)
                   return hn

               def emit_m(l, g, hn):
                   m = mp.tile([128, G * H], dt.float32)
                   if l == 0:
                       for si in range(G):
                           s = g * G + si
                           k = pkcb(s)[0]
                           for kc in range(2):
                               ms = m[:, si * H + kc * 128:
                                      si * H + (kc + 1) * 128]
                               nc.tensor.matmul(
                                   ms,
                                   hn[:, si * H + kc * 128:
                                      si * H + (kc + 1) * 128],
                                   ck[:, 512:640], start=True, stop=False)
                               nc.tensor.matmul(
                                   ms, ck[32 * k:32 * k + 2,
                                          H + kc * 128:H + (kc + 1) * 128],
                                   lx_ap(s), start=False, stop=True)
                   else:
                       # Residual first: one identity matmul per sample PAIR
                       # covers a full psum bank (start=True zeroes the whole
                       # 2KB zero-region, so it must open the group); the A
                       # matmuls then accumulate on top.
                       for p in range(G // 2):
                           nc.tensor.matmul(
                               m[:, p * 512:(p + 1) * 512], ck[:, 640:768],
                               ht[:, (g * G + 2 * p) * H:
                                  (g * G + 2 * p + 2) * H],
                               start=True, stop=False)
                           for sj in range(2):
                               si = 2 * p + sj
                               for kc in range(2):
                                   nc.tensor.matmul(
                                       m[:, si * H + kc * 128:
                                         si * H + (kc + 1) * 128],
                                       hn[:, si * H + kc * 128:
                                          si * H + (kc + 1) * 128],
                                       ck[:, 512:640], start=False,
                                       stop=(sj == 1 and kc == 1))
                   # relu(h + m) -> next h (fp16), one DVE pass per group
                   nc.vector.tensor_scalar_max(
                       ht[:, g * G * H:(g + 1) * G * H], m[:], 0.0)

               if mode == "l0":
                   layer_list = [0]
               elif mode == "l1":
                   nc.vector.memset(ht[:], 0.25)
                   layer_list = [1]
               else:
                   layer_list = list(range(NL))
               for l in layer_list:
                   pend = None
                   for g in range(ng):
                       hn = emit_z(l, g)
                       if pend is not None:
                           emit_m(l, pend[0], pend[1])
                       pend = (g, hn)
                       # warm the classifier weight stream during the last
                       # layer so the transition doesn't stall on HBM
                       if (w1p is not None and l == NL - 1
                               and g >= ng - W1_PREFETCH
                               and w1_issued[0] < W1_PREFETCH):
                           issue_w1(w1p, w1_v)
                   emit_m(l, pend[0], pend[1])

         if mode in ("layers", "l0", "l1"):
             nc.sync.dma_start(out_d[:], ht[:])
         # ---------------- phase 2: classifier ----------------
         ht_v = ht[:].rearrange("p (s c) -> p s c", c=H)  # [128, BC, 256]
         if mode in ("full", "cls"):
           with (
             tc.tile_pool(name="hs", bufs=1) as hsp,
             tc.tile_pool(name="cp", bufs=1, space="PSUM") as cp,
             tc.tile_pool(name="lp", bufs=2, space="PSUM") as lp,
           ):
               hid0 = cp.tile([128, bc], dt.float32, tag="hid0")
               hid1 = cp.tile([128, bc], dt.float32, tag="hid1")
               hids = (hid0, hid1)
               for mc in range(W1_TILES):
                   while w1_issued[0] < min(mc + W1_PREFETCH + 1, W1_TILES):
                       issue_w1(w1p, w1_v)
                   w1t = w1_tiles[mc]
                   for j in range(KB):
                       chunk = mc * KB + j
                       n_idx, hc = chunk // 2, chunk % 2
                       rhs = ht_v[:, :, hc * 128 + n_idx]
                       for kt in range(2):
                           nc.tensor.matmul(
                               hids[kt][:],
                               w1t[:, j * H + kt * 128:j * H + (kt + 1) * 128],
                               rhs, start=(chunk == 0),
                               stop=(chunk == n_chunks - 1))
               w1_tiles.clear()
               w1_issued[0] = 0

               hidsb = hsp.tile([128, 2 * bc], dt.float16)
               for kt in range(2):
                   nc.scalar.activation(
                       hidsb[:, kt * bc:(kt + 1) * bc], hids[kt][:],
                       AF.Relu, bias=cbb[:, kt:kt + 1])

               nt = bc // min(128, bc)
               bw = min(128, bc)
               out_v = out_d[:].rearrange("(t p) j -> p t j", p=bw)
               lgs = hsp.tile([128, nt * OUT], dt.float32, tag="lgs")
               for bt in range(nt):
                   lg = lp.tile([128, OUT], dt.float32)
                   for kc in range(2):
                       nc.tensor.matmul(
                           lg[:bw, :],
                           hidsb[:, kc * bc + bt * bw:kc * bc + (bt + 1) * bw],
                           ck[:, 768 + kc * OUT:768 + (kc + 1) * OUT],
                           start=(kc == 0), stop=(kc == 1))
                   nc.vector.tensor_tensor(
                       lgs[:bw, bt * OUT:(bt + 1) * OUT], lg[:bw, :],
                       cbb[:bw, 2:4], op=mybir.AluOpType.add)
               nc.sync.dma_start(
                   out_v, lgs[:].rearrange("p (t j) -> p t j", j=OUT))
         rep_ctx.close()

    nc.compile()
    return nc


def _get_nc(has_lbias: bool, bc: int = BC):
    key = (has_lbias, bc)
    if key not in _BUILT:
        _BUILT[key] = _build_nc(has_lbias, bc)
    return _BUILT[key]


def _host_arrays(x, edge_attr, enc_w, enc_b, layer_w, layer_b,
                 cls_w1, cls_b1, cls_w2, cls_b2, edge_index):
    f64 = np.float64
    src, dst = edge_index[0], edge_index[1]
    A = np.zeros((N, N), f64)
    np.add.at(A, (dst, src), edge_attr[:, 0].astype(f64))

    u = enc_w[0].astype(f64) @ layer_w[0].astype(f64)
    c = enc_b.astype(f64) @ layer_w[0].astype(f64) + layer_b[0].astype(f64)

    # ck blob [128, 772]: u2c | ew | at_t | eye | w2r
    ck_np = np.zeros((128, 772), np.float32)
    for k in range(3):
        ck_np[32 * k, 0:H] = c                     # u2c: [c; u] pairs
        ck_np[32 * k + 1, 0:H] = u
        for kc in range(2):                        # ew: [enc_b; enc_w] chunks
            ck_np[32 * k, H + kc * 128:H + (kc + 1) * 128] = \
                enc_b[kc * 128:(kc + 1) * 128]
            ck_np[32 * k + 1, H + kc * 128:H + (kc + 1) * 128] = \
                enc_w[0][kc * 128:(kc + 1) * 128]
    ck_np[:, 512:640] = A.T                        # at_t: rhs [m, n] = A[n, m]
    ck_np[:, 640:768] = np.eye(128)                # eye
    ck_np[:, 768:772] = cls_w2.reshape(2, 128, OUT).transpose(1, 0, 2) \
        .reshape(128, 2 * OUT)                     # w2r
    ck_np = ck_np.astype(F16)

    w23_np = layer_w[1:].astype(F16)
    w1_np = cls_w1.astype(F16)
    cbb_np = np.zeros((128, 4), np.float32)
    cbb_np[:, 0:2] = cls_b1.reshape(2, 128).T      # cb1
    cbb_np[:, 2:4] = np.tile(cls_b2, (128, 1))     # cb2

    has_lbias = bool(np.any(layer_b[1:] != 0))
    xb_np = None
    if has_lbias:
        xbt = np.zeros((6, (NL - 1) * H), np.float32)
        for li in range(NL - 1):
            xbt[0::2, li * H:(li + 1) * H] = layer_b[li + 1]
        xb_np = xbt.astype(F16)

    def lhsx_for(x_core):                          # x_core [BC, 128] fp32
        nblk = (BC + 2) // 3
        t = np.zeros((6, nblk * 128), np.float32)
        t[0::2] = 1.0
        for k in range(3):
            # row 2k+1, block cb holds x for sample 3*cb + k
            xs = x_core[k::3].reshape(-1)
            t[2 * k + 1, :xs.size] = xs
        return t.astype(F16)

    shared = {
        "ck": ck_np, "w23": w23_np, "w1": w1_np, "cbb": cbb_np,
    }
    if has_lbias:
        shared["xb23"] = xb_np
    return shared, lhsx_for, has_lbias


def kernel(x, edge_attr, enc_w, enc_b, layer_w, layer_b,
           cls_w1, cls_b1, cls_w2, cls_b2, edge_index):
    args = [np.asarray(a) for a in (
        x, edge_attr, enc_w, enc_b, layer_w, layer_b,
        cls_w1, cls_b1, cls_w2, cls_b2, edge_index)]
    (x, edge_attr, enc_w, enc_b, layer_w, layer_b,
     cls_w1, cls_b1, cls_w2, cls_b2, edge_index) = args

    shared, lhsx_for, has_lbias = _host_arrays(
        x, edge_attr, enc_w, enc_b, layer_w, layer_b,
        cls_w1, cls_b1, cls_w2, cls_b2, edge_index)
    nc = _get_nc(has_lbias)

    in_maps = []
    for cid in range(N_CORES):
        xc = x[cid * BC:(cid + 1) * BC].astype(np.float32)
        m = dict(shared)
        m["lhsx"] = lhsx_for(xc)
        in_maps.append(m)

    res = run_bass_kernel_spmd(nc, in_maps, core_ids=list(range(N_CORES)))
    # slots are processed pair-major: slot j holds original sample perm[j]
    perm = np.concatenate([np.arange(k, BC, 3) for k in range(3)])
    outs = []
    for c in range(N_CORES):
        o = np.empty((BC, OUT), np.float32)
        o[perm] = res.results[c]["logits"]
        outs.append(o)
    return np.concatenate(outs, axis=0).astype(np.float32)


if __name__ == "__main__":
    rng = np.random.default_rng(0)
    ins = {
        "x": rng.standard_normal((B, N), dtype=np.float32),
        "edge_attr": rng.random((4096, 1), dtype=np.float32),
        "enc_w": rng.standard_normal((1, H), dtype=np.float32) * 0.02,
        "enc_b": np.zeros((H,), np.float32),
        "layer_w": rng.standard_normal((NL, H, H), dtype=np.float32) * 0.02,
        "layer_b": np.zeros((NL, H), np.float32),
        "cls_w1": rng.standard_normal((H * N, H), dtype=np.float32) * 0.02,
        "cls_b1": np.zeros((H,), np.float32),
        "cls_w2": rng.standard_normal((H, OUT), dtype=np.float32) * 0.02,
        "cls_b2": np.zeros((OUT,), np.float32),
        "edge_index": rng.integers(0, N, (2, 4096)).astype(np.int32),
    }
    out = kernel(**ins)
    print("kernel ran, out:", out.shape, out.dtype, np.abs(out).max())


# revision 25
# speedup vs baseline: 1.4798x; 1.4798x over previous
"""Trainium2 Bass kernel for nn_CGNN (gnn_message_passing).

Strategy
--------
The per-edge gather/scatter-add over a shared edge list is algebraically a
dense matmul: messages[b] = A @ h_new[b] with A[n, m] = sum_{e: dst=n, src=m}
w_e (A is [128, 128], shared across batch and layers).  The whole network is
then dense matmuls + relu, executed per-sample as [128, 256] tiles:

  - h is stored per-sample TRANSPOSED (hT: feature on partitions, node on
    free dim) in one resident SBUF buffer.  The layer matmul z = h @ W uses
    hT chunks as the stationary operand (lhsT), which re-transposes for free:
    z comes out in normal [node, feature] layout.
  - messages^T = h_new^T-producing matmul: lhsT = h_new (normal layout, from
    the relu eviction of z), rhs = A^T.  Output mT is in hT layout.
  - The residual h + messages is accumulated on the PE itself: an identity
    matmul adds hT into the mT PSUM accumulation; a single DVE
    relu-max eviction then produces the next layer's hT (fp16).
  - The encoder h0 = x*enc_w + enc_b is never materialized: layer 1 uses
    z1 = x (x) u + 1 (x) c  (u = enc_w @ W1, c = enc_b @ W1 + b1, computed on
    host in fp64).  Sample s's [1; x_s] row-pair lives at SBUF partitions
    (32k, 32k+1), k = s%3, col block s//3 of a compact [128, 11008] tile
    (rank-2 stationaries: K=2 matmuls).  h0's residual enters the layer-1
    mT PSUM as a K=2 matmul from [enc_b; enc_w] chunk rows at the same
    partition pair.
  - The classifier hidden = relu(h3.flat @ cls_w1 + b1) reads the resident
    hT buffer with strided APs (no transposes): for each 128-row chunk of
    cls_w1, rhs = hT[h-partitions, batch-strided free].  cls_w1 is streamed
    from HBM in fp16, prefetched during the layer phase, DMAs round-robined
    across the SP/Act/DVE queues.

Data-parallel across 8 cores over the batch axis (256 samples/core).
All matmuls in fp16 (fp32 PSUM accumulation).
"""

import sys

for _p in ("/opt/trn_rl_repo",):
    if _p not in sys.path:
        sys.path.insert(0, _p)

from contextlib import ExitStack

import ml_dtypes
import numpy as np

import concourse.bacc as bacc
import concourse.bass as bass
import concourse.tile as tile
from concourse import mybir
from concourse.bass_utils import run_bass_kernel_spmd

dt = mybir.dt
AF = mybir.ActivationFunctionType
F16 = np.float16

B, N, H, NL, OUT = 2048, 128, 256, 3, 2
N_CORES = 8
BC = B // N_CORES            # samples per core (256)
G = 2                        # samples per elementwise eviction group
NG = BC // G
KB = 8                       # cls_w1 128-row chunks per DMA (512 KB each)
N_CHUNKS = (N * H) // 128    # 256 contraction chunks in the classifier
W1_TILES = N_CHUNKS // KB    # 32
W1_PREFETCH = 4              # w1 tiles DMA'd before the layer loop ends

_BUILT = {}


def _build_nc(has_lbias: bool, bc: int = BC, mode: str = "full",
              repeat: int = 1):
    """Emit the Tile kernel. has_lbias: include the (rare) nonzero
    layer-bias rank-1 accumulations for layers 2..3.
    mode: "full" | "layers" (skip classifier, dump ht) | "cls"
    (skip layers, classifier reads zero-init ht)."""
    ng = bc // G
    n_chunks = N_CHUNKS
    nc = bacc.Bacc("TRN2", target_bir_lowering=False)

    # compact x: sample s -> partition pair (32k, 32k+1), k=s%3, col block
    # s//3.  Row 32k is all-ones, row 32k+1 is x_s.
    lhsx_d = nc.dram_tensor("lhsx", [6, ((bc + 2) // 3) * 128], dt.float16,
                            kind="ExternalInput")
    # packed fp16 const blob: one DMA covers everything the layer phase
    # needs beyond lhsx.  cols: [0:256] u2c ([c; u] at partition pairs),
    # [256:512] ew ([enc_b; enc_w] chunks at pairs), [512:640] at_t,
    # [640:768] eye, [768:772] w2r.
    ck_d = nc.dram_tensor("ck", [128, 772], dt.float16,
                          kind="ExternalInput")
    w23_d = nc.dram_tensor("w23", [NL - 1, H, H], dt.float16,
                           kind="ExternalInput")
    w1_d = nc.dram_tensor("w1", [N * H, H], dt.float16, kind="ExternalInput")
    cbb_d = nc.dram_tensor("cbb", [128, 4], dt.float32,
                           kind="ExternalInput")
    if has_lbias:
        # layer-l bias row at partitions 32k, block l-1
        xb_d = nc.dram_tensor("xb23", [6, (NL - 1) * H], dt.float16,
                              kind="ExternalInput")
    if mode in ("layers", "l0", "l1"):
        out_d = nc.dram_tensor("htdump", [128, bc * H], dt.float16,
                               kind="ExternalOutput")
    else:
        out_d = nc.dram_tensor("logits", [bc, OUT], dt.float32,
                               kind="ExternalOutput")

    with tile.TileContext(nc) as tc, ExitStack() as ctx:
        const = ctx.enter_context(tc.tile_pool(name="const", bufs=1))
        htp = ctx.enter_context(tc.tile_pool(name="ht", bufs=1))

        lhsx = const.tile([128, ((bc + 2) // 3) * 128], dt.float16)
        ck = const.tile([128, 772], dt.float16)
        w23 = const.tile([128, (NL - 1) * 2 * H], dt.float16)
        cbb = const.tile([128, 4], dt.float32)


        # critical path: 4 HWDGE issues total (first z needs lhsx pair 0 +
        # ck; pairs 1/2 right behind)
        nc.sync.dma_start(lhsx[0:2, :], lhsx_d[0:2, :])
        nc.sync.dma_start(ck[:], ck_d[:])
        nc.sync.dma_start(lhsx[32:34, :], lhsx_d[2:4, :])
        nc.sync.dma_start(lhsx[64:66, :], lhsx_d[4:6, :])
        # non-critical consts via SWDGE (Pool) - off the HWDGE device
        for li in range(NL - 1):
            for hc in range(2):
                nc.gpsimd.dma_start(
                    w23[:, (li * 2 + hc) * H:(li * 2 + hc + 1) * H],
                    w23_d[li, hc * 128:(hc + 1) * 128, :])
        nc.gpsimd.dma_start(cbb[:], cbb_d[:])
        if has_lbias:
            xb = const.tile([128, (NL - 1) * H], dt.float16)
            for k in range(3):
                nc.gpsimd.dma_start(xb[32 * k:32 * k + 1, :],
                                    xb_d[2 * k:2 * k + 1, :])

        # resident h (hT layout): sample s chunk hc at cols s*256 + hc*128
        ht = htp.tile([128, bc * H], dt.float16)

        n0 = (bc + 2) // 3
        n1 = (bc + 1) // 3

        def pkcb(j):
            # processing slot j -> (pair k, col block cb); original sample
            # 3*cb + k.  Pair-major so early slots only touch lhsx pair 0.
            if j < n0:
                return 0, j
            if j < n0 + n1:
                return 1, j - n0
            return 2, j - n0 - n1

        def lx_ap(s):
            k, cb = pkcb(s)
            return lhsx[32 * k:32 * k + 2, cb * 128:(cb + 1) * 128]

        if mode == "cls":
            nc.vector.memset(ht[:], 0.5)

        w1_tiles = []          # in-flight w1 tile FIFO for prefetch
        w1_issued = [0]

        def issue_w1(w1p, w1_v):
            mc = w1_issued[0]
            if mc >= W1_TILES:
                return
            w1t = w1p.tile([128, KB * H], dt.float16, name="w1t")
            (nc.sync if mc % 2 == 0 else nc.scalar).dma_start(
                w1t[:].rearrange("p (a k) -> p a k", a=KB),
                w1_v[:, mc * KB:(mc + 1) * KB, :])
            w1_tiles.append(w1t)
            w1_issued[0] = mc + 1

        w1_v = w1_d[:].rearrange("(a p) k -> p a k", p=128)
        w1p = None
        if mode in ("full", "cls"):
            w1p = ctx.enter_context(
                tc.tile_pool(name="w1p", bufs=W1_PREFETCH + 2))

        # ---------------- phase 1: 3 GNN layers ----------------
        for _rep in range(repeat):
         if mode != "cls":
           with (
               tc.tile_pool(name="hn", bufs=5) as hnp,
               tc.tile_pool(name="zp", bufs=8 // G, space="PSUM") as zp,
               tc.tile_pool(name="mp", bufs=8 // G, space="PSUM") as mp,
           ):
               def emit_z(l, g):
                   z = zp.tile([128, G * H], dt.float32)
                   for si in range(G):
                       s = g * G + si
                       zs = z[:, si * H:(si + 1) * H]
                       if l == 0:
                           k = pkcb(s)[0]
                           nc.tensor.matmul(
                               zs, lx_ap(s),
                               ck[32 * k:32 * k + 2, 0:H],
                               start=True, stop=True)
                       else:
                           for hc in range(2):
                               last = (hc == 1) and not has_lbias
                               nc.tensor.matmul(
                                   zs,
                                   ht[:, s * H + hc * 128:s * H + (hc + 1) * 128],
                                   w23[:, ((l - 1) * 2 + hc) * H:
                                       ((l - 1) * 2 + hc + 1) * H],
                                   start=(hc == 0), stop=last)
                           if has_lbias:
                               k = pkcb(s)[0]
                               nc.tensor.matmul(
                                   zs, lx_ap(s)[0:1, :],
                                   xb[32 * k:32 * k + 1,
                                      (l - 1) * H:l * H],
                                   start=False, stop=True)
                   hn = hnp.tile([128, G * H], dt.float16)
                   nc.scalar.activation(hn[:], z[:], AF.Relu)
                   return hn

               def emit_m(l, g, hn):
                   m = mp.tile([128, G * H], dt.float32)
                   if l == 0:
                       for si in range(G):
                           s = g * G + si
                           k = pkcb(s)[0]
                           for kc in range(2):
                               ms = m[:, si * H + kc * 128:
                                      si * H + (kc + 1) * 128]
                               nc.tensor.matmul(
                                   ms,
                                   hn[:, si * H + kc * 128:
                                      si * H + (kc + 1) * 128],
                                   ck[:, 512:640], start=True, stop=False)
                               nc.tensor.matmul(
                                   ms, ck[32 * k:32 * k + 2,
                                          H + kc * 128:H + (kc + 1) * 128],
                                   lx_ap(s), start=False, stop=True)
                   else:
                       # Residual first: one identity matmul per sample PAIR
                       # covers a full psum bank (start=True zeroes the whole
                       # 2KB zero-region, so it must open the group); the A
                       # matmuls then accumulate on top.
                       for p in range(G // 2):
                           nc.tensor.matmul(
                               m[:, p * 512:(p + 1) * 512], ck[:, 640:768],
                               ht[:, (g * G + 2 * p) * H:
                                  (g * G + 2 * p + 2) * H],
                               start=True, stop=False)
                           for sj in range(2):
                               si = 2 * p + sj
                               for kc in range(2):
                                   nc.tensor.matmul(
                                       m[:, si * H + kc * 128:
                                         si * H + (kc + 1) * 128],
                                       hn[:, si * H + kc * 128:
                                          si * H + (kc + 1) * 128],
                                       ck[:, 512:640], start=False,
                                       stop=(sj == 1 and kc == 1))
                   # relu(h + m) -> next h (fp16), one DVE pass per group
                   nc.vector.tensor_scalar_max(
                       ht[:, g * G * H:(g + 1) * G * H], m[:], 0.0)

               if mode == "l0":
                   layer_list = [0]
               elif mode == "l1":
                   nc.vector.memset(ht[:], 0.25)
                   layer_list = [1]
               else:
                   layer_list = list(range(NL))
               # software-pipeline depth 2: two z groups in flight ahead
               # of each m group, covering the Act-eviction latency
               for l in layer_list:
                   pend = []
                   for g in range(ng):
                       hn = emit_z(l, g)
                       pend.append((g, hn))
                       if len(pend) > 2:
                           emit_m(l, *pend.pop(0))
                       # warm the classifier weight stream during the last
                       # layer so the transition doesn't stall on HBM
                       if (w1p is not None and l == NL - 1
                               and g >= ng - W1_PREFETCH
                               and w1_issued[0] < W1_PREFETCH):
                           issue_w1(w1p, w1_v)
                   for pg in pend:
                       emit_m(l, *pg)

         if mode in ("layers", "l0", "l1"):
             nc.sync.dma_start(out_d[:], ht[:])
         # ---------------- phase 2: classifier ----------------
         ht_v = ht[:].rearrange("p (s c) -> p s c", c=H)  # [128, BC, 256]
         if mode in ("full", "cls"):
           with (
             tc.tile_pool(name="hs", bufs=1) as hsp,
             tc.tile_pool(name="cp", bufs=1, space="PSUM") as cp,
             tc.tile_pool(name="lp", bufs=2, space="PSUM") as lp,
           ):
               hid0 = cp.tile([128, bc], dt.float32, tag="hid0")
               hid1 = cp.tile([128, bc], dt.float32, tag="hid1")
               hids = (hid0, hid1)
               for mc in range(W1_TILES):
                   while w1_issued[0] < min(mc + W1_PREFETCH + 1, W1_TILES):
                       issue_w1(w1p, w1_v)
                   w1t = w1_tiles[mc]
                   for j in range(KB):
                       chunk = mc * KB + j
                       n_idx, hc = chunk // 2, chunk % 2
                       rhs = ht_v[:, :, hc * 128 + n_idx]
                       for kt in range(2):
                           nc.tensor.matmul(
                               hids[kt][:],
                               w1t[:, j * H + kt * 128:j * H + (kt + 1) * 128],
                               rhs, start=(chunk == 0),
                               stop=(chunk == n_chunks - 1))
               w1_tiles.clear()
               w1_issued[0] = 0

               hidsb = hsp.tile([128, 2 * bc], dt.float16)
               for kt in range(2):
                   nc.scalar.activation(
                       hidsb[:, kt * bc:(kt + 1) * bc], hids[kt][:],
                       AF.Relu, bias=cbb[:, kt:kt + 1])

               nt = bc // min(128, bc)
               bw = min(128, bc)
               out_v = out_d[:].rearrange("(t p) j -> p t j", p=bw)
               lgs = hsp.tile([128, nt * OUT], dt.float32, tag="lgs")
               for bt in range(nt):
                   lg = lp.tile([128, OUT], dt.float32)
                   for kc in range(2):
                       nc.tensor.matmul(
                           lg[:bw, :],
                           hidsb[:, kc * bc + bt * bw:kc * bc + (bt + 1) * bw],
                           ck[:, 768 + kc * OUT:768 + (kc + 1) * OUT],
                           start=(kc == 0), stop=(kc == 1))
                   nc.vector.tensor_tensor(
                       lgs[:bw, bt * OUT:(bt + 1) * OUT], lg[:bw, :],
                       cbb[:bw, 2:4], op=mybir.AluOpType.add)
               nc.sync.dma_start(
                   out_v, lgs[:].rearrange("p (t j) -> p t j", j=OUT))

    nc.compile()
    return nc


def _get_nc(has_lbias: bool, bc: int = BC):
    key = (has_lbias, bc)
    if key not in _BUILT:
        _BUILT[key] = _build_nc(has_lbias, bc)
    return _BUILT[key]


def _host_arrays(x, edge_attr, enc_w, enc_b, layer_w, layer_b,
                 cls_w1, cls_b1, cls_w2, cls_b2, edge_index):
    f64 = np.float64
    src, dst = edge_index[0], edge_index[1]
    A = np.zeros((N, N), f64)
    np.add.at(A, (dst, src), edge_attr[:, 0].astype(f64))

    u = enc_w[0].astype(f64) @ layer_w[0].astype(f64)
    c = enc_b.astype(f64) @ layer_w[0].astype(f64) + layer_b[0].astype(f64)

    # ck blob [128, 772]: u2c | ew | at_t | eye | w2r
    ck_np = np.zeros((128, 772), np.float32)
    for k in range(3):
        ck_np[32 * k, 0:H] = c                     # u2c: [c; u] pairs
        ck_np[32 * k + 1, 0:H] = u
        for kc in range(2):                        # ew: [enc_b; enc_w] chunks
            ck_np[32 * k, H + kc * 128:H + (kc + 1) * 128] = \
                enc_b[kc * 128:(kc + 1) * 128]
            ck_np[32 * k + 1, H + kc * 128:H + (kc + 1) * 128] = \
                enc_w[0][kc * 128:(kc + 1) * 128]
    ck_np[:, 512:640] = A.T                        # at_t: rhs [m, n] = A[n, m]
    ck_np[:, 640:768] = np.eye(128)                # eye
    ck_np[:, 768:772] = cls_w2.reshape(2, 128, OUT).transpose(1, 0, 2) \
        .reshape(128, 2 * OUT)                     # w2r
    ck_np = ck_np.astype(F16)

    w23_np = layer_w[1:].astype(F16)
    w1_np = cls_w1.astype(F16)
    cbb_np = np.zeros((128, 4), np.float32)
    cbb_np[:, 0:2] = cls_b1.reshape(2, 128).T      # cb1
    cbb_np[:, 2:4] = np.tile(cls_b2, (128, 1))     # cb2

    has_lbias = bool(np.any(layer_b[1:] != 0))
    xb_np = None
    if has_lbias:
        xbt = np.zeros((6, (NL - 1) * H), np.float32)
        for li in range(NL - 1):
            xbt[0::2, li * H:(li + 1) * H] = layer_b[li + 1]
        xb_np = xbt.astype(F16)

    def lhsx_for(x_core):                          # x_core [BC, 128] fp32
        nblk = (BC + 2) // 3
        t = np.zeros((6, nblk * 128), np.float32)
        t[0::2] = 1.0
        for k in range(3):
            # row 2k+1, block cb holds x for sample 3*cb + k
            xs = x_core[k::3].reshape(-1)
            t[2 * k + 1, :xs.size] = xs
        return t.astype(F16)

    shared = {
        "ck": ck_np, "w23": w23_np, "w1": w1_np, "cbb": cbb_np,
    }
    if has_lbias:
        shared["xb23"] = xb_np
    return shared, lhsx_for, has_lbias


def kernel(x, edge_attr, enc_w, enc_b, layer_w, layer_b,
           cls_w1, cls_b1, cls_w2, cls_b2, edge_index):
    args = [np.asarray(a) for a in (
        x, edge_attr, enc_w, enc_b, layer_w, layer_b,
        cls_w1, cls_b1, cls_w2, cls_b2, edge_index)]
    (x, edge_attr, enc_w, enc_b, layer_w, layer_b,
     cls_w1, cls_b1, cls_w2, cls_b2, edge_index) = args

    shared, lhsx_for, has_lbias = _host_arrays(
        x, edge_attr, enc_w, enc_b, layer_w, layer_b,
        cls_w1, cls_b1, cls_w2, cls_b2, edge_index)
    nc = _get_nc(has_lbias)

    in_maps = []
    for cid in range(N_CORES):
        xc = x[cid * BC:(cid + 1) * BC].astype(np.float32)
        m = dict(shared)
        m["lhsx"] = lhsx_for(xc)
        in_maps.append(m)

    res = run_bass_kernel_spmd(nc, in_maps, core_ids=list(range(N_CORES)))
    # slots are processed pair-major: slot j holds original sample perm[j]
    perm = np.concatenate([np.arange(k, BC, 3) for k in range(3)])
    outs = []
    for c in range(N_CORES):
        o = np.empty((BC, OUT), np.float32)
        o[perm] = res.results[c]["logits"]
        outs.append(o)
    return np.concatenate(outs, axis=0).astype(np.float32)


if __name__ == "__main__":
    rng = np.random.default_rng(0)
    ins = {
        "x": rng.standard_normal((B, N), dtype=np.float32),
        "edge_attr": rng.random((4096, 1), dtype=np.float32),
        "enc_w": rng.standard_normal((1, H), dtype=np.float32) * 0.02,
        "enc_b": np.zeros((H,), np.float32),
        "layer_w": rng.standard_normal((NL, H, H), dtype=np.float32) * 0.02,
        "layer_b": np.zeros((NL, H), np.float32),
        "cls_w1": rng.standard_normal((H * N, H), dtype=np.float32) * 0.02,
        "cls_b1": np.zeros((H,), np.float32),
        "cls_w2": rng.standard_normal((H, OUT), dtype=np.float32) * 0.02,
        "cls_b2": np.zeros((OUT,), np.float32),
        "edge_index": rng.integers(0, N, (2, 4096)).astype(np.int32),
    }
    out = kernel(**ins)
    print("kernel ran, out:", out.shape, out.dtype, np.abs(out).max())
